# revision 32
# baseline (speedup 1.0000x reference)
"""Trainium2 Bass kernel for the 2-layer GAT node-classification head.

The reference reads only h2[mask_idx] and x[mask_idx], so the computation
collapses to mask_idx's 2-hop in-neighborhood: V1 = sources of mask's
in-edges (incl. self-loop), S2 = in-edges of V1, U = unique sources of S2.

Head-sharded across the 8 cores (H1 == 8 heads): head h's entire layer-1
GAT (attention softmax + value aggregation + W1 GEMM + elu) touches only
W1[:, h*768:(h+1)*768] and is independent of the other heads, so core h
streams just its 590KB fp8 W1 slice (vs 4.7MB replicated) and contracts
its elu'd h1 dims with the layer-2 weight folds [w2fold | Ws2 | Wd2].
The per-core partial h2f [v1n, 4] sums across cores at gather time; the
remaining layer-2 segment-softmax over mask's s1n in-edges plus the
classifier is ~100 flops applied to the gathered sums on the host.

Per-core program:
  1. cst16: one bf16 constants tensor (edge-gathered x chunks, per-head
     att folds, scatter one-hots, x[U] packed [64 x 384], layer-2 fold
     slice, x[m]/fc fold for the oxm term), SP/HWDGE DMA.
  2. w1: [128, 6*768] fp8 head slice (x64 prescale), single SP DMA whose
     transfer overlaps the attention chain.
  3. out: all results land on partition 0 ([1, 4*v1n+2]: per-v h2f
     4-blocks + oxm) and leave via a PREPARE_ONLY dma_scatter_add whose
     descriptors are generated early on the Pool engine; the final
     trigger_dma skips the ~1.3us HWDGE+DGE latency of a plain DMA.
     The scatter target is zeroed by a tiny Pool-issued DMA at start.
Attention runs while w1 streams: per-edge logits via 12 accumulating
matmuls (edge-gathered x against folded Ws/Wd), exp(lrelu) = max(exp(x),
exp(0.2x)), per-group normalization via one-hot matmuls + reciprocal,
aggregate-first xagg, then the 36-block fp8 GEMM accumulates as w1 lands.
elu' = elu+1 = max(x,0) + exp(min(x,0)); the -1 is folded on the host;
the min part runs on Pool so DVE/ACT/Pool work in parallel on the tail.
"""

import numpy as np
import ml_dtypes

import concourse.mybir as mybir
import concourse.tile as tile
from concourse import bacc
from concourse.bass_utils import run_bass_kernel_spmd

NCORES = 8
P = 128
C = 768          # input feature dim
H1 = 8           # layer-1 heads
OUT = 768        # per-head feature dim
KC = C // P      # 6 contraction chunks of 128
UP = 32          # padded unique-source rows (PE partition-base alignment)
W1SCALE = 64.0   # fp8 prescale for W1 (clears e4m3 subnormals)

f32 = mybir.dt.float32
bf16 = mybir.dt.bfloat16
fp8 = mybir.dt.float8e4
np_bf16 = ml_dtypes.bfloat16
np_fp8 = ml_dtypes.float8_e4m3


# ---------------------------------------------------------------- host graph
def _preprocess(edge_index, mask_idx, n_nodes):
    """Extract the 2-hop in-neighborhood of mask_idx (with multiplicity)."""
    ei = np.asarray(edge_index).astype(np.int64)
    m = int(np.asarray(mask_idx))
    src_all = np.concatenate([ei[0], np.arange(n_nodes, dtype=np.int64)])
    dst_all = np.concatenate([ei[1], np.arange(n_nodes, dtype=np.int64)])

    s1_pos = np.nonzero(dst_all == m)[0]          # in-edges of m (incl self)
    s1_src = src_all[s1_pos].tolist()
    v1 = list(dict.fromkeys(s1_src))              # unique sources
    v1n = len(v1)
    assert v1n * 2 * UP <= P, f"mask in-degree too large: {v1n}"

    groups = [src_all[np.nonzero(dst_all == v)[0]].tolist() for v in v1]
    gmax = max(len(g) for g in groups)
    s2p = v1n * gmax
    assert s2p <= P, f"edge tile too large: {s2p}"

    u = list(dict.fromkeys([s for g in groups for s in g]))
    un = len(u)
    assert un <= UP, f"too many unique 2-hop sources: {un}"
    urow = {node: r for r, node in enumerate(u)}

    meta = dict(v1n=v1n, gmax=gmax)
    host = dict(m=m, v1=v1, u=u, urow=urow, groups=groups, s1_src=s1_src)
    return meta, host


def _lay16(meta):
    """Column layout of the bf16 packed-constants tensor."""
    v1n, gmax = meta["v1n"], meta["gmax"]
    s2p = v1n * gmax
    pieces = [
        ("xe", P, KC * 2 * s2p),     # x[src_e]/x[dst_e] chunks, edge cols
        ("wsd", P, KC * 2),          # per-head [Ws|Wd] fold chunks
        ("xu64", 64, (KC // 2) * P), # x[U] packed (c%2 -> row half)
        ("u01r", s2p, 2 * UP),       # edge -> (r, u) one-hot, both halves
        ("ones64", s2p, 2 * UP),     # all-ones (denominator expand)
        ("mask4", s2p, 2 * v1n),     # edge -> (r, v) group mask
        ("w2f", P, KC * 4),          # per-head [w2fold|Ws2|Wd2] chunks
        ("xm", P, KC),               # x[m] chunks
        ("wfb", P, KC * 2),          # wf_bot = (fc_w @ cls_w)[768:] chunks
        ("bias3s", 1, 2),
        ("one11", 1, 1),
    ]
    lay, off = {}, 0
    for name, rows, cols in pieces:
        lay[name] = (rows, off, cols)
        off += cols
    return lay, off


def _chunked(w):
    """[K, N] -> [128, (K//128)*N] chunk-major free layout."""
    k, n = w.shape
    assert k % P == 0
    return np.ascontiguousarray(
        w.reshape(k // P, P, n).transpose(1, 0, 2).reshape(P, (k // P) * n))


# ---------------------------------------------------------------- bass build
def _build(meta):
    v1n, gmax = meta["v1n"], meta["gmax"]
    s2p = v1n * gmax
    vp = max(v1n, 2)
    lay16, cw16 = _lay16(meta)

    nc = bacc.Bacc("TRN2", target_bir_lowering=False, debug=False,
                   enable_asserts=False, num_devices=NCORES)

    d_cst16 = nc.dram_tensor("cst16", [P, cw16], bf16, kind="ExternalInput")
    d_w1a = nc.dram_tensor("w1a", [P, (KC - 1) * OUT], fp8,
                           kind="ExternalInput")
    d_w1b = nc.dram_tensor("w1b", [P, OUT], fp8, kind="ExternalInput")
    d_res = nc.dram_tensor("res", [1, 4 * v1n + 2], f32,
                           kind="ExternalOutput")

    with tile.TileContext(nc) as tc:
        with (
            tc.tile_pool(name="const", bufs=1) as cpool,
            tc.tile_pool(name="sbuf", bufs=1) as sb,
            tc.tile_pool(name="big", bufs=1) as bigp,
            tc.tile_pool(name="ps", bufs=1, space="PSUM") as ps,
        ):
            cst16 = cpool.tile([P, cw16], bf16, tag="cst16")
            nc.sync.dma_start(out=cst16[:], in_=d_cst16[:])
            w1a_sb = bigp.tile([P, (KC - 1) * OUT], fp8, tag="w1a")
            nc.sync.dma_start(out=w1a_sb[:], in_=d_w1a[:])
            w1b_sb = bigp.tile([P, OUT], fp8, tag="w1b")
            nc.sync.dma_start(out=w1b_sb[:], in_=d_w1b[:])

            def cv(name):
                rows, off, cols = lay16[name]
                return cst16[0:rows, off:off + cols]

            xe_v = cv("xe")
            wsd_v = cv("wsd")
            xu64_v = cv("xu64")
            u01r_v = cv("u01r")
            ones64_v = cv("ones64")
            mask4_v = cv("mask4")
            w2f_v = cv("w2f")
            xm_v = cv("xm")
            wfb_v = cv("wfb")
            bias3s_v = cv("bias3s")
            one11_v = cv("one11")

            # PSUM tiles (each its own bank -> independent accum groups)
            lg_ps = ps.tile([s2p, 1], f32, tag="lg")
            wuvu_ps = ps.tile([2 * UP, 2 * v1n], f32, tag="wuvu")
            den_ps = ps.tile([2 * UP, 2 * v1n], f32, tag="den")
            xagg_ps = ps.tile([P, KC * v1n], f32, tag="xagg")
            agg_ps = ps.tile([P, KC * v1n], f32, tag="agg")
            h2f_ps = ps.tile([1, 4 * v1n + 2], f32, tag="h2f")
            oxm_ps = h2f_ps[0:1, 4 * v1n:4 * v1n + 2]

            # zero rows / cols for bank-zeroing matmuls
            zrow = cpool.tile([1, P], bf16, tag="zrow")
            nc.vector.memset(zrow[:], 0.0)
            zcols = cpool.tile([1, max(KC * v1n, 4 * v1n + 2)], bf16,
                               tag="zcols")
            nc.vector.memset(zcols[:], 0.0)
            nc.tensor.matmul(out=h2f_ps[:], lhsT=zrow[:, 0:1],
                             rhs=zcols[:, 0:4 * v1n + 2],
                             start=True, stop=False, skip_group_check=True)

            # ---- oxm = x[m] @ wf_bot + bias3s (off critical path) ----
            for c in range(KC):
                nc.tensor.matmul(out=oxm_ps, lhsT=xm_v[:, c:c + 1],
                                 rhs=wfb_v[:, 2 * c:2 * c + 2],
                                 start=False, stop=False,
                                 skip_group_check=True)
            nc.tensor.matmul(out=oxm_ps, lhsT=one11_v, rhs=bias3s_v,
                             start=False, stop=True, skip_group_check=True)

            # ---- per-edge logits for this head: [s2p, 1] ----
            for c in range(KC):
                for d in range(2):
                    nc.tensor.matmul(
                        out=lg_ps[:],
                        lhsT=xe_v[:, (c * 2 + d) * s2p:(c * 2 + d + 1) * s2p],
                        rhs=wsd_v[:, c * 2 + d:c * 2 + d + 1],
                        start=(c == 0 and d == 0),
                        stop=(c == KC - 1 and d == 1))

            # exp(lrelu(x)) = max(exp(x), exp(0.2x))
            ea_sb = sb.tile([s2p, 1], f32, tag="ea")
            nc.scalar.activation(out=ea_sb[:], in_=lg_ps[:],
                                 func=mybir.ActivationFunctionType.Exp)
            eb_sb = sb.tile([s2p, 1], f32, tag="eb")
            nc.scalar.activation(out=eb_sb[:], in_=lg_ps[:],
                                 func=mybir.ActivationFunctionType.Exp,
                                 scale=0.2)
            ee_sb = sb.tile([s2p, 1], f32, tag="ee")
            nc.vector.tensor_tensor(out=ee_sb[:], in0=ea_sb[:], in1=eb_sb[:],
                                    op=mybir.AluOpType.max)
            # group-masked copies: eev[e, (r, v)] = ee[e] * [grp(e) == v]
            eev_sb = sb.tile([s2p, 2 * v1n], bf16, tag="eev")
            nc.vector.tensor_tensor(
                out=eev_sb[:], in0=ee_sb[:].to_broadcast([s2p, 2 * v1n]),
                in1=mask4_v, op=mybir.AluOpType.mult)

            # wuvu[(r,u), (r,v)] = sum_{e in grp v, src u} ee;  den = grp sum
            nc.tensor.matmul(out=wuvu_ps[:], lhsT=u01r_v, rhs=eev_sb[:],
                             start=True, stop=True)
            nc.tensor.matmul(out=den_ps[:], lhsT=ones64_v, rhs=eev_sb[:],
                             start=True, stop=True)
            rec_sb = sb.tile([2 * UP, 2 * v1n], f32, tag="rec")
            nc.vector.reciprocal(out=rec_sb[:], in_=den_ps[:])
            wuv_sb = sb.tile([2 * UP, 2 * v1n], bf16, tag="wuv")
            nc.vector.tensor_tensor(out=wuv_sb[:], in0=wuvu_ps[:],
                                    in1=rec_sb[:], op=mybir.AluOpType.mult)

            # xagg[c-part, (c, v)] = sum_u x[u] * wuv[u, v]
            for c in range(KC):
                r = c % 2
                xl = xu64_v[32 * r:32 * r + 32,
                            (c // 2) * P:(c // 2 + 1) * P]
                for v in range(v1n):
                    nc.tensor.matmul(
                        out=xagg_ps[:, c * v1n + v:c * v1n + v + 1],
                        lhsT=xl,
                        rhs=wuv_sb[32 * r:32 * r + 32,
                                   r * v1n + v:r * v1n + v + 1],
                        start=True, stop=True)
            xagg8 = sb.tile([P, KC * v1n], fp8, tag="xagg8")
            nc.vector.tensor_copy(out=xagg8[:], in_=xagg_ps[:])

            # ---- big GEMM: agg[f-part, (fb, v)] += W1c^T @ xagg8_c ----
            # one start=True matmul zeroes the whole bank; per-block starts
            # would wipe neighbors (PSUM zero region is coarse).  W1 is split
            # c0..c4 / c5 so only the last chunk's matmuls trail its DMA.
            nc.tensor.matmul(out=agg_ps[:], lhsT=zrow[:],
                             rhs=zcols[:, 0:KC * v1n],
                             start=True, stop=False, skip_group_check=True)
            for c in range(KC - 1):
                for fb in range(KC):
                    nc.tensor.matmul(
                        out=agg_ps[:, fb * v1n:(fb + 1) * v1n],
                        lhsT=w1a_sb[:, c * OUT + fb * P:
                                    c * OUT + (fb + 1) * P],
                        rhs=xagg8[:, c * v1n:(c + 1) * v1n],
                        start=False, stop=False,
                        skip_group_check=True)
            for fb in range(KC):
                nc.tensor.matmul(
                    out=agg_ps[:, fb * v1n:(fb + 1) * v1n],
                    lhsT=w1b_sb[:, fb * P:(fb + 1) * P],
                    rhs=xagg8[:, (KC - 1) * v1n:KC * v1n],
                    start=False, stop=True,
                    skip_group_check=True)

            # elu'(x) = elu(x)+1 = max(x,0) + min(exp(x),1); x = agg/64.
            # ACT exps agg straight out of PSUM (|x| < ~10 by construction,
            # no overflow) while DVE does the max; one DVE min caps it.
            ef_sb = sb.tile([P, KC * v1n], f32, tag="ef")
            nc.scalar.activation(out=ef_sb[:], in_=agg_ps[:],
                                 func=mybir.ActivationFunctionType.Exp,
                                 scale=1.0 / W1SCALE)
            t1_sb = sb.tile([P, KC * v1n], bf16, tag="t1")
            nc.vector.tensor_scalar(out=t1_sb[:], in0=agg_ps[:],
                                    scalar1=1.0 / W1SCALE, scalar2=0.0,
                                    op0=mybir.AluOpType.mult,
                                    op1=mybir.AluOpType.max)
            ee2_sb = sb.tile([P, KC * v1n], bf16, tag="ee2")
            nc.vector.tensor_scalar(out=ee2_sb[:], in0=ef_sb[:],
                                    scalar1=1.0, scalar2=None,
                                    op0=mybir.AluOpType.min,
                                    op1=mybir.AluOpType.bypass)

            # ---- h2f partials on partition 0: [1, (v, 4)] ----
            for i, t in enumerate((t1_sb, ee2_sb)):
                for v in range(v1n):
                    for fb in range(KC):
                        nc.tensor.matmul(
                            out=h2f_ps[0:1, v * 4:(v + 1) * 4],
                            lhsT=t[:, fb * v1n + v:fb * v1n + v + 1],
                            rhs=w2f_v[:, fb * 4:(fb + 1) * 4],
                            start=False,
                            stop=(i == 1 and fb == KC - 1),
                            skip_group_check=True)

            res_sb = sb.tile([1, 4 * v1n + 2], f32, tag="res")
            nc.vector.tensor_copy(out=res_sb[:], in_=h2f_ps[:])
            nc.sync.dma_start(out=d_res[:], in_=res_sb[:])

    nc.compile()
    return nc


_CACHE = {}


def _get_nc(meta):
    key = repr(sorted(meta.items()))
    if key not in _CACHE:
        _CACHE[key] = _build(meta)
    return _CACHE[key]


def _prepare(**inputs):
    x = np.asarray(inputs["x"], np.float32)
    n_nodes = x.shape[0]
    meta, host = _preprocess(inputs["edge_index"], inputs["mask_idx"], n_nodes)
    v1n, gmax = meta["v1n"], meta["gmax"]
    s2p = v1n * gmax
    groups, urow, v1, u, m = (host["groups"], host["urow"], host["v1"],
                              host["u"], host["m"])

    W1 = np.asarray(inputs["W1"], np.float32)
    att_s1 = np.asarray(inputs["att_src1"], np.float32)
    att_d1 = np.asarray(inputs["att_dst1"], np.float32)
    W2 = np.asarray(inputs["W2"], np.float32)
    att_s2 = np.asarray(inputs["att_src2"], np.float32)
    att_d2 = np.asarray(inputs["att_dst2"], np.float32)
    b2 = np.asarray(inputs["b2"], np.float32)
    fc_w = np.asarray(inputs["fc_w"], np.float32)
    fc_b = np.asarray(inputs["fc_b"], np.float32)
    cls_w = np.asarray(inputs["cls_w"], np.float32)
    cls_b = np.asarray(inputs["cls_b"], np.float32)
    assert not np.any(np.asarray(inputs["b1"])), "b1 != 0 unsupported"

    # weight-weight folds
    Ws1 = np.einsum("chf,hf->ch", W1.reshape(C, H1, OUT), att_s1)   # [C, H1]
    Wd1 = np.einsum("chf,hf->ch", W1.reshape(C, H1, OUT), att_d1)
    Ws2 = W2 @ att_s2[0]                                            # [H1*OUT]
    Wd2 = W2 @ att_d2[0]
    wf = fc_w @ cls_w                                               # [1536, 2]
    wf_top, wf_bot = wf[:OUT], wf[OUT:]
    w2fold = W2 @ wf_top                                            # [6144, 2]
    bias3s = (b2 @ wf_top + fc_b @ cls_w + cls_b).reshape(1, 2)

    # edge-slot layout: group g occupies cols [g*gmax, g*gmax + len(g))
    edges = []                                        # (slot, grp, src)
    for g, srcs in enumerate(groups):
        for j, s in enumerate(srcs):
            edges.append((g * gmax + j, g, s))

    xe = np.zeros((P, KC * 2 * s2p), np.float32)
    for c in range(KC):
        xs = x[:, c * P:(c + 1) * P]
        for e, g, s in edges:
            xe[:, (c * 2 + 0) * s2p + e] = xs[s]
            xe[:, (c * 2 + 1) * s2p + e] = xs[v1[g]]

    xu64 = np.zeros((64, (KC // 2) * P), np.float32)
    for c in range(KC):
        for r, node in enumerate(u):
            xu64[r + 32 * (c % 2), (c // 2) * P:(c // 2 + 1) * P] = \
                x[node, c * P:(c + 1) * P]

    u01r = np.zeros((s2p, 2 * UP), np.float32)
    mask4 = np.zeros((s2p, 2 * v1n), np.float32)
    for e, g, s in edges:
        for r in range(2):
            u01r[e, r * UP + urow[s]] = 1.0
            mask4[e, r * v1n + g] = 1.0
    ones64 = np.ones((s2p, 2 * UP), np.float32)

    lay16, cw16 = _lay16(meta)
    base16 = np.zeros((P, cw16), np.float32)

    def fill(a, name, arr):
        rows, off, cols = lay16[name]
        assert arr.shape == (rows, cols), (name, arr.shape, (rows, cols))
        a[0:rows, off:off + cols] = arr

    fill(base16, "xe", xe)
    fill(base16, "xu64", xu64)
    fill(base16, "u01r", u01r)
    fill(base16, "ones64", ones64)
    fill(base16, "mask4", mask4)
    fill(base16, "xm", np.ascontiguousarray(x[m].reshape(KC, P).T))
    fill(base16, "wfb", _chunked(np.ascontiguousarray(wf_bot)))
    fill(base16, "bias3s", bias3s)
    fill(base16, "one11", np.ones((1, 1), np.float32))

    in_maps = []
    for h in range(NCORES):
        a = base16.copy()
        wsd = np.zeros((P, KC * 2), np.float32)
        for c in range(KC):
            wsd[:, c * 2 + 0] = Ws1[c * P:(c + 1) * P, h]
            wsd[:, c * 2 + 1] = Wd1[c * P:(c + 1) * P, h]
        fill(a, "wsd", wsd)
        w2f4 = np.concatenate(
            [w2fold[h * OUT:(h + 1) * OUT],
             Ws2[h * OUT:(h + 1) * OUT, None],
             Wd2[h * OUT:(h + 1) * OUT, None]], axis=1)     # [768, 4]
        fill(a, "w2f", _chunked(np.ascontiguousarray(w2f4)))
        w1h = np.ascontiguousarray(W1[:, h * OUT:(h + 1) * OUT]) * W1SCALE
        w1c = _chunked(w1h).astype(np_fp8)
        in_maps.append({
            "cst16": a.astype(np_bf16),
            "w1a": np.ascontiguousarray(w1c[:, 0:(KC - 1) * OUT]),
            "w1b": np.ascontiguousarray(w1c[:, (KC - 1) * OUT:KC * OUT]),
        })

    tail = dict(
        v1n=v1n, m=m, v1row={v: r for r, v in enumerate(v1)},
        s1_src=host["s1_src"],
        colsum2=w2fold.sum(axis=0).astype(np.float64),
        sws2=float(Ws2.sum()), swd2=float(Wd2.sum()),
    )
    return meta, in_maps, tail


def make_in_maps(**inputs):
    meta, in_maps, _ = _prepare(**inputs)
    return meta, in_maps


def _host_tail(tail, h2f, oxm):
    """Layer-2 segment softmax over mask's in-edges + classifier add."""
    v1row, m = tail["v1row"], tail["m"]
    vs = h2f[:, 0:2] - tail["colsum2"]            # helu'-1 fold
    a2s = h2f[:, 2] - tail["sws2"]
    a2d_m = h2f[v1row[m], 3] - tail["swd2"]
    lg = np.array([a2s[v1row[s]] for s in tail["s1_src"]], np.float64) + a2d_m
    lg = np.where(lg > 0, lg, 0.2 * lg)
    e = np.exp(lg - lg.max())
    alpha = e / (e.sum() + 1e-16)
    h2top = alpha @ np.stack([vs[v1row[s]] for s in tail["s1_src"]])
    return (h2top + oxm).reshape(1, 2).astype(np.float32)


def kernel(**inputs):
    meta, in_maps, tail = _prepare(**inputs)
    nc = _get_nc(meta)
    res = run_bass_kernel_spmd(nc, in_maps, core_ids=list(range(NCORES)))
    v1n = meta["v1n"]
    parts = [np.asarray(r["res"], np.float64).reshape(4 * v1n + 2)
             for r in res.results]
    h2f = sum(p[0:4 * v1n] for p in parts).reshape(v1n, 4)
    oxm = parts[0][4 * v1n:4 * v1n + 2]
    return _host_tail(tail, h2f, oxm)


# revision 33
# speedup vs baseline: 1.0050x; 1.0050x over previous
"""Trainium2 Bass kernel for the 2-layer GAT node-classification head.

The reference reads only h2[mask_idx] and x[mask_idx], so the computation
collapses to mask_idx's 2-hop in-neighborhood: V1 = sources of mask's
in-edges (incl. self-loop), S2 = in-edges of V1, U = unique sources of S2.

Head-sharded across the 8 cores (H1 == 8 heads): head h's entire layer-1
GAT (attention softmax + value aggregation + W1 GEMM + elu) touches only
W1[:, h*768:(h+1)*768] and is independent of the other heads, so core h
streams just its 590KB fp8 W1 slice (vs 4.7MB replicated) and contracts
its elu'd h1 dims with the layer-2 weight folds [w2fold | Ws2 | Wd2].
The per-core partial h2f [v1n, 4] sums across cores at gather time; the
remaining layer-2 segment-softmax over mask's s1n in-edges plus the
classifier is ~100 flops applied to the gathered sums on the host.

Per-core program:
  1. cst16: one bf16 constants tensor (edge-gathered x chunks, per-head
     att folds, scatter one-hots, x[U] packed [64 x 384], layer-2 fold
     slice, x[m]/fc fold for the oxm term), SP/HWDGE DMA.
  2. w1: [128, 6*768] fp8 head slice (x64 prescale), single SP DMA whose
     transfer overlaps the attention chain.
  3. out: all results land on partition 0 ([1, 4*v1n+2]: per-v h2f
     4-blocks + oxm) and leave via a PREPARE_ONLY dma_scatter_add whose
     descriptors are generated early on the Pool engine; the final
     trigger_dma skips the ~1.3us HWDGE+DGE latency of a plain DMA.
     The scatter target is zeroed by a tiny Pool-issued DMA at start.
Attention runs while w1 streams: per-edge logits via 12 accumulating
matmuls (edge-gathered x against folded Ws/Wd), exp(lrelu) = max(exp(x),
exp(0.2x)), per-group normalization via one-hot matmuls + reciprocal,
aggregate-first xagg, then the 36-block fp8 GEMM accumulates as w1 lands.
elu' = elu+1 = max(x,0) + exp(min(x,0)); the -1 is folded on the host;
the min part runs on Pool so DVE/ACT/Pool work in parallel on the tail.
"""

import numpy as np
import ml_dtypes

import concourse.mybir as mybir
import concourse.tile as tile
from concourse import bacc
from concourse.bass_utils import run_bass_kernel_spmd

NCORES = 8
P = 128
C = 768          # input feature dim
H1 = 8           # layer-1 heads
OUT = 768        # per-head feature dim
KC = C // P      # 6 contraction chunks of 128
UP = 32          # padded unique-source rows (PE partition-base alignment)
W1SCALE = 64.0   # fp8 prescale for W1 (clears e4m3 subnormals)

f32 = mybir.dt.float32
bf16 = mybir.dt.bfloat16
fp8 = mybir.dt.float8e4
np_bf16 = ml_dtypes.bfloat16
np_fp8 = ml_dtypes.float8_e4m3


# ---------------------------------------------------------------- host graph
def _preprocess(edge_index, mask_idx, n_nodes):
    """Extract the 2-hop in-neighborhood of mask_idx (with multiplicity)."""
    ei = np.asarray(edge_index).astype(np.int64)
    m = int(np.asarray(mask_idx))
    src_all = np.concatenate([ei[0], np.arange(n_nodes, dtype=np.int64)])
    dst_all = np.concatenate([ei[1], np.arange(n_nodes, dtype=np.int64)])

    s1_pos = np.nonzero(dst_all == m)[0]          # in-edges of m (incl self)
    s1_src = src_all[s1_pos].tolist()
    v1 = list(dict.fromkeys(s1_src))              # unique sources
    v1n = len(v1)
    assert v1n * 2 * UP <= P, f"mask in-degree too large: {v1n}"

    groups = [src_all[np.nonzero(dst_all == v)[0]].tolist() for v in v1]
    gmax = max(len(g) for g in groups)
    s2p = v1n * gmax
    assert s2p <= P, f"edge tile too large: {s2p}"

    u = list(dict.fromkeys([s for g in groups for s in g]))
    un = len(u)
    assert un <= UP, f"too many unique 2-hop sources: {un}"
    urow = {node: r for r, node in enumerate(u)}

    meta = dict(v1n=v1n, gmax=gmax)
    host = dict(m=m, v1=v1, u=u, urow=urow, groups=groups, s1_src=s1_src)
    return meta, host


def _lay16(meta):
    """Column layout of the bf16 packed-constants tensor."""
    v1n, gmax = meta["v1n"], meta["gmax"]
    s2p = v1n * gmax
    pieces = [
        ("xe", P, KC * 2 * s2p),     # x[src_e]/x[dst_e] chunks, edge cols
        ("wsd", P, KC * 2),          # per-head [Ws|Wd] fold chunks
        ("xu64", 64, (KC // 2) * P), # x[U] packed (c%2 -> row half)
        ("u01r", s2p, 2 * UP),       # edge -> (r, u) one-hot, both halves
        ("ones64", s2p, 2 * UP),     # all-ones (denominator expand)
        ("mask4", s2p, 2 * v1n),     # edge -> (r, v) group mask
        ("w2f", P, KC * 4),          # per-head [w2fold|Ws2|Wd2] chunks
        ("xm", P, KC),               # x[m] chunks
        ("wfb", P, KC * 2),          # wf_bot = (fc_w @ cls_w)[768:] chunks
        ("bias3s", 1, 2),
        ("one11", 1, 1),
    ]
    lay, off = {}, 0
    for name, rows, cols in pieces:
        lay[name] = (rows, off, cols)
        off += cols
    return lay, off


def _chunked(w):
    """[K, N] -> [128, (K//128)*N] chunk-major free layout."""
    k, n = w.shape
    assert k % P == 0
    return np.ascontiguousarray(
        w.reshape(k // P, P, n).transpose(1, 0, 2).reshape(P, (k // P) * n))


# ---------------------------------------------------------------- bass build
def _build(meta):
    v1n, gmax = meta["v1n"], meta["gmax"]
    s2p = v1n * gmax
    vp = max(v1n, 2)
    lay16, cw16 = _lay16(meta)

    nc = bacc.Bacc("TRN2", target_bir_lowering=False, debug=False,
                   enable_asserts=False, num_devices=NCORES)

    d_cst16 = nc.dram_tensor("cst16", [P, cw16], bf16, kind="ExternalInput")
    d_w1a = nc.dram_tensor("w1a", [P, (KC - 1) * OUT], fp8,
                           kind="ExternalInput")
    d_w1b = nc.dram_tensor("w1b", [P, OUT], fp8, kind="ExternalInput")
    d_res = nc.dram_tensor("res", [1, 4 * v1n + 2], f32,
                           kind="ExternalOutput")

    with tile.TileContext(nc) as tc:
        with (
            tc.tile_pool(name="const", bufs=1) as cpool,
            tc.tile_pool(name="sbuf", bufs=1) as sb,
            tc.tile_pool(name="big", bufs=1) as bigp,
            tc.tile_pool(name="ps", bufs=1, space="PSUM") as ps,
        ):
            cst16 = cpool.tile([P, cw16], bf16, tag="cst16")
            nc.sync.dma_start(out=cst16[:], in_=d_cst16[:])
            w1a_sb = bigp.tile([P, (KC - 1) * OUT], fp8, tag="w1a")
            nc.sync.dma_start(out=w1a_sb[:], in_=d_w1a[:])
            w1b_sb = bigp.tile([P, OUT], fp8, tag="w1b")
            nc.sync.dma_start(out=w1b_sb[:], in_=d_w1b[:])

            def cv(name):
                rows, off, cols = lay16[name]
                return cst16[0:rows, off:off + cols]

            xe_v = cv("xe")
            wsd_v = cv("wsd")
            xu64_v = cv("xu64")
            u01r_v = cv("u01r")
            ones64_v = cv("ones64")
            mask4_v = cv("mask4")
            w2f_v = cv("w2f")
            xm_v = cv("xm")
            wfb_v = cv("wfb")
            bias3s_v = cv("bias3s")
            one11_v = cv("one11")

            # PSUM tiles (each its own bank -> independent accum groups)
            lg_ps = ps.tile([s2p, 1], f32, tag="lg")
            wuvu_ps = ps.tile([2 * UP, 2 * v1n], f32, tag="wuvu")
            den_ps = ps.tile([2 * UP, 2 * v1n], f32, tag="den")
            xagg_ps = ps.tile([P, KC * v1n], f32, tag="xagg")
            agg_ps = ps.tile([P, KC * v1n], f32, tag="agg")
            h2f_ps = ps.tile([1, 4 * v1n + 2], f32, tag="h2f")
            oxm_ps = h2f_ps[0:1, 4 * v1n:4 * v1n + 2]

            # zero rows / cols for bank-zeroing matmuls
            zrow = cpool.tile([1, P], bf16, tag="zrow")
            nc.vector.memset(zrow[:], 0.0)
            zcols = cpool.tile([1, max(KC * v1n, 4 * v1n + 2)], bf16,
                               tag="zcols")
            nc.vector.memset(zcols[:], 0.0)
            nc.tensor.matmul(out=h2f_ps[:], lhsT=zrow[:, 0:1],
                             rhs=zcols[:, 0:4 * v1n + 2],
                             start=True, stop=False, skip_group_check=True)

            # ---- oxm = x[m] @ wf_bot + bias3s (off critical path) ----
            for c in range(KC):
                nc.tensor.matmul(out=oxm_ps, lhsT=xm_v[:, c:c + 1],
                                 rhs=wfb_v[:, 2 * c:2 * c + 2],
                                 start=False, stop=False,
                                 skip_group_check=True)
            nc.tensor.matmul(out=oxm_ps, lhsT=one11_v, rhs=bias3s_v,
                             start=False, stop=True, skip_group_check=True)

            # ---- per-edge logits for this head: [s2p, 1] ----
            for c in range(KC):
                for d in range(2):
                    nc.tensor.matmul(
                        out=lg_ps[:],
                        lhsT=xe_v[:, (c * 2 + d) * s2p:(c * 2 + d + 1) * s2p],
                        rhs=wsd_v[:, c * 2 + d:c * 2 + d + 1],
                        start=(c == 0 and d == 0),
                        stop=(c == KC - 1 and d == 1))

            # exp(lrelu(x)) = max(exp(x), exp(0.2x))
            ea_sb = sb.tile([s2p, 1], f32, tag="ea")
            nc.scalar.activation(out=ea_sb[:], in_=lg_ps[:],
                                 func=mybir.ActivationFunctionType.Exp)
            eb_sb = sb.tile([s2p, 1], f32, tag="eb")
            nc.scalar.activation(out=eb_sb[:], in_=lg_ps[:],
                                 func=mybir.ActivationFunctionType.Exp,
                                 scale=0.2)
            ee_sb = sb.tile([s2p, 1], f32, tag="ee")
            nc.vector.tensor_tensor(out=ee_sb[:], in0=ea_sb[:], in1=eb_sb[:],
                                    op=mybir.AluOpType.max)
            # group-masked copies: eev[e, (r, v)] = ee[e] * [grp(e) == v]
            eev_sb = sb.tile([s2p, 2 * v1n], bf16, tag="eev")
            nc.vector.tensor_tensor(
                out=eev_sb[:], in0=ee_sb[:].to_broadcast([s2p, 2 * v1n]),
                in1=mask4_v, op=mybir.AluOpType.mult)

            # wuvu[(r,u), (r,v)] = sum_{e in grp v, src u} ee;  den = grp sum
            nc.tensor.matmul(out=wuvu_ps[:], lhsT=u01r_v, rhs=eev_sb[:],
                             start=True, stop=True)
            nc.tensor.matmul(out=den_ps[:], lhsT=ones64_v, rhs=eev_sb[:],
                             start=True, stop=True)
            rec_sb = sb.tile([2 * UP, 2 * v1n], f32, tag="rec")
            nc.vector.reciprocal(out=rec_sb[:], in_=den_ps[:])
            wuv_sb = sb.tile([2 * UP, 2 * v1n], bf16, tag="wuv")
            nc.vector.tensor_tensor(out=wuv_sb[:], in0=wuvu_ps[:],
                                    in1=rec_sb[:], op=mybir.AluOpType.mult)

            # xagg[c-part, (c, v)] = sum_u x[u] * wuv[u, v]
            for c in range(KC):
                r = c % 2
                xl = xu64_v[32 * r:32 * r + 32,
                            (c // 2) * P:(c // 2 + 1) * P]
                for v in range(v1n):
                    nc.tensor.matmul(
                        out=xagg_ps[:, c * v1n + v:c * v1n + v + 1],
                        lhsT=xl,
                        rhs=wuv_sb[32 * r:32 * r + 32,
                                   r * v1n + v:r * v1n + v + 1],
                        start=True, stop=True)
            xagg8 = sb.tile([P, KC * v1n], fp8, tag="xagg8")
            nc.vector.tensor_copy(out=xagg8[:], in_=xagg_ps[:])

            # ---- big GEMM: agg[f-part, (fb, v)] += W1c^T @ xagg8_c ----
            # one start=True matmul zeroes the whole bank; per-block starts
            # would wipe neighbors (PSUM zero region is coarse).  W1 is split
            # c0..c4 / c5 so only the last chunk's matmuls trail its DMA.
            nc.tensor.matmul(out=agg_ps[:], lhsT=zrow[:],
                             rhs=zcols[:, 0:KC * v1n],
                             start=True, stop=False, skip_group_check=True)
            for c in range(KC - 1):
                for fb in range(KC):
                    nc.tensor.matmul(
                        out=agg_ps[:, fb * v1n:(fb + 1) * v1n],
                        lhsT=w1a_sb[:, c * OUT + fb * P:
                                    c * OUT + (fb + 1) * P],
                        rhs=xagg8[:, c * v1n:(c + 1) * v1n],
                        start=False, stop=False,
                        skip_group_check=True)
            for fb in range(KC):
                nc.tensor.matmul(
                    out=agg_ps[:, fb * v1n:(fb + 1) * v1n],
                    lhsT=w1b_sb[:, fb * P:(fb + 1) * P],
                    rhs=xagg8[:, (KC - 1) * v1n:KC * v1n],
                    start=False, stop=True,
                    skip_group_check=True)

            # elu'(x) = elu(x)+1 = max(x,0) + exp(min(x,0)); x = agg/64.
            # min first: it feeds the exp; max runs while ACT does the exp.
            t0_sb = sb.tile([P, KC * v1n], f32, tag="t0")
            nc.vector.tensor_scalar(out=t0_sb[:], in0=agg_ps[:],
                                    scalar1=1.0 / W1SCALE, scalar2=0.0,
                                    op0=mybir.AluOpType.mult,
                                    op1=mybir.AluOpType.min)
            t1_sb = sb.tile([P, KC * v1n], bf16, tag="t1")
            nc.vector.tensor_scalar(out=t1_sb[:], in0=agg_ps[:],
                                    scalar1=1.0 / W1SCALE, scalar2=0.0,
                                    op0=mybir.AluOpType.mult,
                                    op1=mybir.AluOpType.max)
            ee2_sb = sb.tile([P, KC * v1n], bf16, tag="ee2")
            nc.scalar.activation(out=ee2_sb[:], in_=t0_sb[:],
                                 func=mybir.ActivationFunctionType.Exp)

            # ---- h2f partials on partition 0: [1, (v, 4)] ----
            for i, t in enumerate((t1_sb, ee2_sb)):
                for v in range(v1n):
                    for fb in range(KC):
                        nc.tensor.matmul(
                            out=h2f_ps[0:1, v * 4:(v + 1) * 4],
                            lhsT=t[:, fb * v1n + v:fb * v1n + v + 1],
                            rhs=w2f_v[:, fb * 4:(fb + 1) * 4],
                            start=False,
                            stop=(i == 1 and fb == KC - 1),
                            skip_group_check=True)

            res_sb = sb.tile([1, 4 * v1n + 2], f32, tag="res")
            nc.vector.tensor_copy(out=res_sb[:], in_=h2f_ps[:])
            nc.sync.dma_start(out=d_res[:], in_=res_sb[:])

    nc.compile()
    return nc


_CACHE = {}


def _get_nc(meta):
    key = repr(sorted(meta.items()))
    if key not in _CACHE:
        _CACHE[key] = _build(meta)
    return _CACHE[key]


def _prepare(**inputs):
    x = np.asarray(inputs["x"], np.float32)
    n_nodes = x.shape[0]
    meta, host = _preprocess(inputs["edge_index"], inputs["mask_idx"], n_nodes)
    v1n, gmax = meta["v1n"], meta["gmax"]
    s2p = v1n * gmax
    groups, urow, v1, u, m = (host["groups"], host["urow"], host["v1"],
                              host["u"], host["m"])

    W1 = np.asarray(inputs["W1"], np.float32)
    att_s1 = np.asarray(inputs["att_src1"], np.float32)
    att_d1 = np.asarray(inputs["att_dst1"], np.float32)
    W2 = np.asarray(inputs["W2"], np.float32)
    att_s2 = np.asarray(inputs["att_src2"], np.float32)
    att_d2 = np.asarray(inputs["att_dst2"], np.float32)
    b2 = np.asarray(inputs["b2"], np.float32)
    fc_w = np.asarray(inputs["fc_w"], np.float32)
    fc_b = np.asarray(inputs["fc_b"], np.float32)
    cls_w = np.asarray(inputs["cls_w"], np.float32)
    cls_b = np.asarray(inputs["cls_b"], np.float32)
    assert not np.any(np.asarray(inputs["b1"])), "b1 != 0 unsupported"

    # weight-weight folds
    Ws1 = np.einsum("chf,hf->ch", W1.reshape(C, H1, OUT), att_s1)   # [C, H1]
    Wd1 = np.einsum("chf,hf->ch", W1.reshape(C, H1, OUT), att_d1)
    Ws2 = W2 @ att_s2[0]                                            # [H1*OUT]
    Wd2 = W2 @ att_d2[0]
    wf = fc_w @ cls_w                                               # [1536, 2]
    wf_top, wf_bot = wf[:OUT], wf[OUT:]
    w2fold = W2 @ wf_top                                            # [6144, 2]
    bias3s = (b2 @ wf_top + fc_b @ cls_w + cls_b).reshape(1, 2)

    # edge-slot layout: group g occupies cols [g*gmax, g*gmax + len(g))
    edges = []                                        # (slot, grp, src)
    for g, srcs in enumerate(groups):
        for j, s in enumerate(srcs):
            edges.append((g * gmax + j, g, s))

    xe = np.zeros((P, KC * 2 * s2p), np.float32)
    for c in range(KC):
        xs = x[:, c * P:(c + 1) * P]
        for e, g, s in edges:
            xe[:, (c * 2 + 0) * s2p + e] = xs[s]
            xe[:, (c * 2 + 1) * s2p + e] = xs[v1[g]]

    xu64 = np.zeros((64, (KC // 2) * P), np.float32)
    for c in range(KC):
        for r, node in enumerate(u):
            xu64[r + 32 * (c % 2), (c // 2) * P:(c // 2 + 1) * P] = \
                x[node, c * P:(c + 1) * P]

    u01r = np.zeros((s2p, 2 * UP), np.float32)
    mask4 = np.zeros((s2p, 2 * v1n), np.float32)
    for e, g, s in edges:
        for r in range(2):
            u01r[e, r * UP + urow[s]] = 1.0
            mask4[e, r * v1n + g] = 1.0
    ones64 = np.ones((s2p, 2 * UP), np.float32)

    lay16, cw16 = _lay16(meta)
    base16 = np.zeros((P, cw16), np.float32)

    def fill(a, name, arr):
        rows, off, cols = lay16[name]
        assert arr.shape == (rows, cols), (name, arr.shape, (rows, cols))
        a[0:rows, off:off + cols] = arr

    fill(base16, "xe", xe)
    fill(base16, "xu64", xu64)
    fill(base16, "u01r", u01r)
    fill(base16, "ones64", ones64)
    fill(base16, "mask4", mask4)
    fill(base16, "xm", np.ascontiguousarray(x[m].reshape(KC, P).T))
    fill(base16, "wfb", _chunked(np.ascontiguousarray(wf_bot)))
    fill(base16, "bias3s", bias3s)
    fill(base16, "one11", np.ones((1, 1), np.float32))

    in_maps = []
    for h in range(NCORES):
        a = base16.copy()
        wsd = np.zeros((P, KC * 2), np.float32)
        for c in range(KC):
            wsd[:, c * 2 + 0] = Ws1[c * P:(c + 1) * P, h]
            wsd[:, c * 2 + 1] = Wd1[c * P:(c + 1) * P, h]
        fill(a, "wsd", wsd)
        w2f4 = np.concatenate(
            [w2fold[h * OUT:(h + 1) * OUT],
             Ws2[h * OUT:(h + 1) * OUT, None],
             Wd2[h * OUT:(h + 1) * OUT, None]], axis=1)     # [768, 4]
        fill(a, "w2f", _chunked(np.ascontiguousarray(w2f4)))
        w1h = np.ascontiguousarray(W1[:, h * OUT:(h + 1) * OUT]) * W1SCALE
        w1c = _chunked(w1h).astype(np_fp8)
        in_maps.append({
            "cst16": a.astype(np_bf16),
            "w1a": np.ascontiguousarray(w1c[:, 0:(KC - 1) * OUT]),
            "w1b": np.ascontiguousarray(w1c[:, (KC - 1) * OUT:KC * OUT]),
        })

    tail = dict(
        v1n=v1n, m=m, v1row={v: r for r, v in enumerate(v1)},
        s1_src=host["s1_src"],
        colsum2=w2fold.sum(axis=0).astype(np.float64),
        sws2=float(Ws2.sum()), swd2=float(Wd2.sum()),
    )
    return meta, in_maps, tail


def make_in_maps(**inputs):
    meta, in_maps, _ = _prepare(**inputs)
    return meta, in_maps


def _host_tail(tail, h2f, oxm):
    """Layer-2 segment softmax over mask's in-edges + classifier add."""
    v1row, m = tail["v1row"], tail["m"]
    vs = h2f[:, 0:2] - tail["colsum2"]            # helu'-1 fold
    a2s = h2f[:, 2] - tail["sws2"]
    a2d_m = h2f[v1row[m], 3] - tail["swd2"]
    lg = np.array([a2s[v1row[s]] for s in tail["s1_src"]], np.float64) + a2d_m
    lg = np.where(lg > 0, lg, 0.2 * lg)
    e = np.exp(lg - lg.max())
    alpha = e / (e.sum() + 1e-16)
    h2top = alpha @ np.stack([vs[v1row[s]] for s in tail["s1_src"]])
    return (h2top + oxm).reshape(1, 2).astype(np.float32)


def kernel(**inputs):
    meta, in_maps, tail = _prepare(**inputs)
    nc = _get_nc(meta)
    res = run_bass_kernel_spmd(nc, in_maps, core_ids=list(range(NCORES)))
    v1n = meta["v1n"]
    parts = [np.asarray(r["res"], np.float64).reshape(4 * v1n + 2)
             for r in res.results]
    h2f = sum(p[0:4 * v1n] for p in parts).reshape(v1n, 4)
    oxm = parts[0][4 * v1n:4 * v1n + 2]
    return _host_tail(tail, h2f, oxm)


# revision 35
# speedup vs baseline: 1.0333x; 1.0282x over previous
"""Trainium2 Bass kernel for the 2-layer GAT node-classification head.

The reference reads only h2[mask_idx] and x[mask_idx], so the computation
collapses to mask_idx's 2-hop in-neighborhood: V1 = sources of mask's
in-edges (incl. self-loop), S2 = in-edges of V1, U = unique sources of S2.

Head-sharded across the 8 cores (H1 == 8 heads): head h's entire layer-1
GAT (attention softmax + value aggregation + W1 GEMM + elu) touches only
W1[:, h*768:(h+1)*768] and is independent of the other heads, so core h
streams just its 590KB fp8 W1 slice (vs 4.7MB replicated) and contracts
its elu'd h1 dims with the layer-2 weight folds [w2fold | Ws2 | Wd2].
The per-core partial h2f [v1n, 4] sums across cores at gather time; the
remaining layer-2 segment-softmax over mask's s1n in-edges plus the
classifier is ~100 flops applied to the gathered sums on the host.

Per-core program (3 input DMAs, 1 output DMA):
  1. cst16: one bf16 constants tensor (edge-gathered x chunks, per-head
     att folds, softmax one-hots, x[U] packed [64 x 384], layer-2 fold
     slice, x[m]/fc fold for the oxm term), first on the wire.
  2. w1a/w1b: [128, 6*768] fp8 head slice (x64 prescale) split c0..c4/c5
     so only the last chunk's six matmuls trail the final DMA semaphore;
     the transfer overlaps the attention chain.
  3. out: all results land on partition 0 ([1, 4*v1n+2]: per-v h2f
     4-blocks + oxm, accumulated in one PSUM bank) -> one DVE copy ->
     one small DMA.
Attention runs while w1 streams: per-edge logits via 12 accumulating
matmuls (edge-gathered x against folded Ws/Wd), exp(lrelu) = max(exp(x),
exp(0.2x)), per-group normalization via one-hot matmuls + reciprocal,
aggregate-first xagg, then the 36-block fp8 GEMM accumulates as w1 lands.
elu' = elu+1 = max(x,0) + exp(min(x,0)); the -1 is folded on the host.
"""

import numpy as np
import ml_dtypes

import concourse.mybir as mybir
import concourse.tile as tile
from concourse import bacc
from concourse.bass_utils import run_bass_kernel_spmd

NCORES = 8
P = 128
C = 768          # input feature dim
H1 = 8           # layer-1 heads
OUT = 768        # per-head feature dim
KC = C // P      # 6 contraction chunks of 128
UP = 32          # padded unique-source rows (PE partition-base alignment)
W1SCALE = 64.0   # fp8 prescale for W1 (clears e4m3 subnormals)

f32 = mybir.dt.float32
bf16 = mybir.dt.bfloat16
fp8 = mybir.dt.float8e4
np_bf16 = ml_dtypes.bfloat16
np_fp8 = ml_dtypes.float8_e4m3


# ---------------------------------------------------------------- host graph
def _preprocess(edge_index, mask_idx, n_nodes):
    """Extract the 2-hop in-neighborhood of mask_idx (with multiplicity)."""
    ei = np.asarray(edge_index).astype(np.int64)
    m = int(np.asarray(mask_idx))
    src_all = np.concatenate([ei[0], np.arange(n_nodes, dtype=np.int64)])
    dst_all = np.concatenate([ei[1], np.arange(n_nodes, dtype=np.int64)])

    s1_pos = np.nonzero(dst_all == m)[0]          # in-edges of m (incl self)
    s1_src = src_all[s1_pos].tolist()
    v1 = list(dict.fromkeys(s1_src))              # unique sources
    v1n = len(v1)
    assert v1n * 2 * UP <= P, f"mask in-degree too large: {v1n}"

    groups = [src_all[np.nonzero(dst_all == v)[0]].tolist() for v in v1]
    gmax = max(len(g) for g in groups)
    s2p = v1n * gmax
    assert s2p <= P, f"edge tile too large: {s2p}"

    u = list(dict.fromkeys([s for g in groups for s in g]))
    un = len(u)
    assert un <= UP, f"too many unique 2-hop sources: {un}"
    urow = {node: r for r, node in enumerate(u)}

    meta = dict(v1n=v1n, gmax=gmax)
    host = dict(m=m, v1=v1, u=u, urow=urow, groups=groups, s1_src=s1_src)
    return meta, host


def _lay16(meta):
    """Column layout of the bf16 packed-constants tensor."""
    v1n, gmax = meta["v1n"], meta["gmax"]
    s2p = v1n * gmax
    pieces = [
        ("xe", P, KC * 2 * s2p),     # x[src_e]/x[dst_e] chunks, edge cols
        ("wsd", P, KC * 2),          # per-head [Ws|Wd] fold chunks
        ("xu64", 64, (KC // 2) * P), # x[U] packed (c%2 -> row half)
        ("u01r", s2p, 2 * UP),       # edge -> (r, u) one-hot, both halves
        ("ones64", s2p, 2 * UP),     # all-ones (denominator expand)
        ("mask4", s2p, 2 * v1n),     # edge -> (r, v) group mask
        ("w2f", P, KC * 4),          # per-head [w2fold|Ws2|Wd2] chunks
        ("xm", P, KC),               # x[m] chunks
        ("wfb", P, KC * 2),          # wf_bot = (fc_w @ cls_w)[768:] chunks
        ("bias3s", 1, 2),
        ("one11", 1, 1),
    ]
    lay, off = {}, 0
    for name, rows, cols in pieces:
        lay[name] = (rows, off, cols)
        off += cols
    return lay, off


def _chunked(w):
    """[K, N] -> [128, (K//128)*N] chunk-major free layout."""
    k, n = w.shape
    assert k % P == 0
    return np.ascontiguousarray(
        w.reshape(k // P, P, n).transpose(1, 0, 2).reshape(P, (k // P) * n))


# ---------------------------------------------------------------- bass build
def _build(meta):
    v1n, gmax = meta["v1n"], meta["gmax"]
    s2p = v1n * gmax
    vp = max(v1n, 2)
    lay16, cw16 = _lay16(meta)

    nc = bacc.Bacc("TRN2", target_bir_lowering=False, debug=False,
                   enable_asserts=False, num_devices=NCORES)

    d_cst16 = nc.dram_tensor("cst16", [P, cw16], bf16, kind="ExternalInput")
    d_w1a = nc.dram_tensor("w1a", [P, (KC - 1) * OUT], fp8,
                           kind="ExternalInput")
    d_w1b = nc.dram_tensor("w1b", [P, OUT], fp8, kind="ExternalInput")
    d_res = nc.dram_tensor("res", [1, 4 * v1n + 2], f32,
                           kind="ExternalOutput")

    with tile.TileContext(nc) as tc:
        with (
            tc.tile_pool(name="const", bufs=1) as cpool,
            tc.tile_pool(name="sbuf", bufs=1) as sb,
            tc.tile_pool(name="big", bufs=1) as bigp,
            tc.tile_pool(name="ps", bufs=1, space="PSUM") as ps,
        ):
            cst16 = cpool.tile([P, cw16], bf16, tag="cst16")
            nc.sync.dma_start(out=cst16[:], in_=d_cst16[:])
            w1a_sb = bigp.tile([P, (KC - 1) * OUT], fp8, tag="w1a")
            nc.sync.dma_start(out=w1a_sb[:], in_=d_w1a[:])
            w1b_sb = bigp.tile([P, OUT], fp8, tag="w1b")
            nc.sync.dma_start(out=w1b_sb[:], in_=d_w1b[:])

            def cv(name):
                rows, off, cols = lay16[name]
                return cst16[0:rows, off:off + cols]

            xe_v = cv("xe")
            wsd_v = cv("wsd")
            xu64_v = cv("xu64")
            u01r_v = cv("u01r")
            ones64_v = cv("ones64")
            mask4_v = cv("mask4")
            w2f_v = cv("w2f")
            xm_v = cv("xm")
            wfb_v = cv("wfb")
            bias3s_v = cv("bias3s")
            one11_v = cv("one11")

            # PSUM tiles (each its own bank -> independent accum groups)
            lg_ps = ps.tile([s2p, 1], f32, tag="lg")
            wuvu_ps = ps.tile([2 * UP, 2 * v1n], f32, tag="wuvu")
            den_ps = ps.tile([2 * UP, 2 * v1n], f32, tag="den")
            xagg_ps = ps.tile([P, KC * v1n], f32, tag="xagg")
            agg_ps = ps.tile([P, KC * v1n], f32, tag="agg")
            h2f_ps = ps.tile([1, 4 * v1n + 2], f32, tag="h2f")
            oxm_ps = h2f_ps[0:1, 4 * v1n:4 * v1n + 2]

            # zero rows / cols for bank-zeroing matmuls
            zrow = cpool.tile([1, P], bf16, tag="zrow")
            nc.vector.memset(zrow[:], 0.0)
            zcols = cpool.tile([1, max(KC * v1n, 4 * v1n + 2)], bf16,
                               tag="zcols")
            nc.vector.memset(zcols[:], 0.0)
            nc.tensor.matmul(out=h2f_ps[:], lhsT=zrow[:, 0:1],
                             rhs=zcols[:, 0:4 * v1n + 2],
                             start=True, stop=False, skip_group_check=True)

            # ---- oxm = x[m] @ wf_bot + bias3s (off critical path) ----
            for c in range(KC):
                nc.tensor.matmul(out=oxm_ps, lhsT=xm_v[:, c:c + 1],
                                 rhs=wfb_v[:, 2 * c:2 * c + 2],
                                 start=False, stop=False,
                                 skip_group_check=True)
            nc.tensor.matmul(out=oxm_ps, lhsT=one11_v, rhs=bias3s_v,
                             start=False, stop=True, skip_group_check=True)

            # ---- per-edge logits for this head: [s2p, 1] ----
            for c in range(KC):
                for d in range(2):
                    nc.tensor.matmul(
                        out=lg_ps[:],
                        lhsT=xe_v[:, (c * 2 + d) * s2p:(c * 2 + d + 1) * s2p],
                        rhs=wsd_v[:, c * 2 + d:c * 2 + d + 1],
                        start=(c == 0 and d == 0),
                        stop=(c == KC - 1 and d == 1))

            # exp(lrelu(x)) = max(exp(x), exp(0.2x))
            ea_sb = sb.tile([s2p, 1], f32, tag="ea")
            nc.scalar.activation(out=ea_sb[:], in_=lg_ps[:],
                                 func=mybir.ActivationFunctionType.Exp)
            eb_sb = sb.tile([s2p, 1], f32, tag="eb")
            nc.scalar.activation(out=eb_sb[:], in_=lg_ps[:],
                                 func=mybir.ActivationFunctionType.Exp,
                                 scale=0.2)
            ee_sb = sb.tile([s2p, 1], f32, tag="ee")
            nc.vector.tensor_tensor(out=ee_sb[:], in0=ea_sb[:], in1=eb_sb[:],
                                    op=mybir.AluOpType.max)
            # group-masked copies: eev[e, (r, v)] = ee[e] * [grp(e) == v]
            eev_sb = sb.tile([s2p, 2 * v1n], bf16, tag="eev")
            nc.vector.tensor_tensor(
                out=eev_sb[:], in0=ee_sb[:].to_broadcast([s2p, 2 * v1n]),
                in1=mask4_v, op=mybir.AluOpType.mult)

            # wuvu[(r,u), (r,v)] = sum_{e in grp v, src u} ee;  den = grp sum
            nc.tensor.matmul(out=wuvu_ps[:], lhsT=u01r_v, rhs=eev_sb[:],
                             start=True, stop=True)
            nc.tensor.matmul(out=den_ps[:], lhsT=ones64_v, rhs=eev_sb[:],
                             start=True, stop=True)
            wuv_sb = sb.tile([2 * UP, 2 * v1n], bf16, tag="wuv")
            nc.vector.tensor_tensor(out=wuv_sb[:], in0=wuvu_ps[:],
                                    in1=den_ps[:], op=mybir.AluOpType.divide)

            # xagg[c-part, (c, v)] = sum_u x[u] * wuv[u, v]
            for c in range(KC):
                r = c % 2
                xl = xu64_v[32 * r:32 * r + 32,
                            (c // 2) * P:(c // 2 + 1) * P]
                for v in range(v1n):
                    nc.tensor.matmul(
                        out=xagg_ps[:, c * v1n + v:c * v1n + v + 1],
                        lhsT=xl,
                        rhs=wuv_sb[32 * r:32 * r + 32,
                                   r * v1n + v:r * v1n + v + 1],
                        start=True, stop=True)
            xagg8 = sb.tile([P, KC * v1n], fp8, tag="xagg8")
            nc.vector.tensor_copy(out=xagg8[:], in_=xagg_ps[:])

            # ---- big GEMM: agg[f-part, (fb, v)] += W1c^T @ xagg8_c ----
            # one start=True matmul zeroes the whole bank; per-block starts
            # would wipe neighbors (PSUM zero region is coarse).  W1 is split
            # c0..c4 / c5 so only the last chunk's matmuls trail its DMA.
            nc.tensor.matmul(out=agg_ps[:], lhsT=zrow[:],
                             rhs=zcols[:, 0:KC * v1n],
                             start=True, stop=False, skip_group_check=True)
            for c in range(KC - 1):
                for fb in range(KC):
                    nc.tensor.matmul(
                        out=agg_ps[:, fb * v1n:(fb + 1) * v1n],
                        lhsT=w1a_sb[:, c * OUT + fb * P:
                                    c * OUT + (fb + 1) * P],
                        rhs=xagg8[:, c * v1n:(c + 1) * v1n],
                        start=False, stop=False,
                        skip_group_check=True)
            for fb in range(KC):
                nc.tensor.matmul(
                    out=agg_ps[:, fb * v1n:(fb + 1) * v1n],
                    lhsT=w1b_sb[:, fb * P:(fb + 1) * P],
                    rhs=xagg8[:, (KC - 1) * v1n:KC * v1n],
                    start=False, stop=True,
                    skip_group_check=True)

            # elu'(x) = elu(x)+1 = max(x,0) + exp(min(x,0)); x = agg/64.
            # min first: it feeds the exp; max runs while ACT does the exp.
            t0_sb = sb.tile([P, KC * v1n], f32, tag="t0")
            nc.vector.tensor_scalar(out=t0_sb[:], in0=agg_ps[:],
                                    scalar1=1.0 / W1SCALE, scalar2=0.0,
                                    op0=mybir.AluOpType.mult,
                                    op1=mybir.AluOpType.min)
            t1_sb = sb.tile([P, KC * v1n], bf16, tag="t1")
            nc.vector.tensor_scalar(out=t1_sb[:], in0=agg_ps[:],
                                    scalar1=1.0 / W1SCALE, scalar2=0.0,
                                    op0=mybir.AluOpType.mult,
                                    op1=mybir.AluOpType.max)
            ee2_sb = sb.tile([P, KC * v1n], bf16, tag="ee2")
            nc.scalar.activation(out=ee2_sb[:], in_=t0_sb[:],
                                 func=mybir.ActivationFunctionType.Exp)

            # ---- h2f partials on partition 0: [1, (v, 4)] ----
            for i, t in enumerate((t1_sb, ee2_sb)):
                for v in range(v1n):
                    for fb in range(KC):
                        nc.tensor.matmul(
                            out=h2f_ps[0:1, v * 4:(v + 1) * 4],
                            lhsT=t[:, fb * v1n + v:fb * v1n + v + 1],
                            rhs=w2f_v[:, fb * 4:(fb + 1) * 4],
                            start=False,
                            stop=(i == 1 and fb == KC - 1),
                            skip_group_check=True)

            res_sb = sb.tile([1, 4 * v1n + 2], f32, tag="res")
            nc.vector.tensor_copy(out=res_sb[:], in_=h2f_ps[:])
            nc.sync.dma_start(out=d_res[:], in_=res_sb[:])

    nc.compile()
    return nc


_CACHE = {}


def _get_nc(meta):
    key = repr(sorted(meta.items()))
    if key not in _CACHE:
        _CACHE[key] = _build(meta)
    return _CACHE[key]


def _prepare(**inputs):
    x = np.asarray(inputs["x"], np.float32)
    n_nodes = x.shape[0]
    meta, host = _preprocess(inputs["edge_index"], inputs["mask_idx"], n_nodes)
    v1n, gmax = meta["v1n"], meta["gmax"]
    s2p = v1n * gmax
    groups, urow, v1, u, m = (host["groups"], host["urow"], host["v1"],
                              host["u"], host["m"])

    W1 = np.asarray(inputs["W1"], np.float32)
    att_s1 = np.asarray(inputs["att_src1"], np.float32)
    att_d1 = np.asarray(inputs["att_dst1"], np.float32)
    W2 = np.asarray(inputs["W2"], np.float32)
    att_s2 = np.asarray(inputs["att_src2"], np.float32)
    att_d2 = np.asarray(inputs["att_dst2"], np.float32)
    b2 = np.asarray(inputs["b2"], np.float32)
    fc_w = np.asarray(inputs["fc_w"], np.float32)
    fc_b = np.asarray(inputs["fc_b"], np.float32)
    cls_w = np.asarray(inputs["cls_w"], np.float32)
    cls_b = np.asarray(inputs["cls_b"], np.float32)
    assert not np.any(np.asarray(inputs["b1"])), "b1 != 0 unsupported"

    # weight-weight folds
    Ws1 = np.einsum("chf,hf->ch", W1.reshape(C, H1, OUT), att_s1)   # [C, H1]
    Wd1 = np.einsum("chf,hf->ch", W1.reshape(C, H1, OUT), att_d1)
    Ws2 = W2 @ att_s2[0]                                            # [H1*OUT]
    Wd2 = W2 @ att_d2[0]
    wf = fc_w @ cls_w                                               # [1536, 2]
    wf_top, wf_bot = wf[:OUT], wf[OUT:]
    w2fold = W2 @ wf_top                                            # [6144, 2]
    bias3s = (b2 @ wf_top + fc_b @ cls_w + cls_b).reshape(1, 2)

    # edge-slot layout: group g occupies cols [g*gmax, g*gmax + len(g))
    edges = []                                        # (slot, grp, src)
    for g, srcs in enumerate(groups):
        for j, s in enumerate(srcs):
            edges.append((g * gmax + j, g, s))

    xe = np.zeros((P, KC * 2 * s2p), np.float32)
    for c in range(KC):
        xs = x[:, c * P:(c + 1) * P]
        for e, g, s in edges:
            xe[:, (c * 2 + 0) * s2p + e] = xs[s]
            xe[:, (c * 2 + 1) * s2p + e] = xs[v1[g]]

    xu64 = np.zeros((64, (KC // 2) * P), np.float32)
    for c in range(KC):
        for r, node in enumerate(u):
            xu64[r + 32 * (c % 2), (c // 2) * P:(c // 2 + 1) * P] = \
                x[node, c * P:(c + 1) * P]

    u01r = np.zeros((s2p, 2 * UP), np.float32)
    mask4 = np.zeros((s2p, 2 * v1n), np.float32)
    for e, g, s in edges:
        for r in range(2):
            u01r[e, r * UP + urow[s]] = 1.0
            mask4[e, r * v1n + g] = 1.0
    ones64 = np.ones((s2p, 2 * UP), np.float32)

    lay16, cw16 = _lay16(meta)
    base16 = np.zeros((P, cw16), np.float32)

    def fill(a, name, arr):
        rows, off, cols = lay16[name]
        assert arr.shape == (rows, cols), (name, arr.shape, (rows, cols))
        a[0:rows, off:off + cols] = arr

    fill(base16, "xe", xe)
    fill(base16, "xu64", xu64)
    fill(base16, "u01r", u01r)
    fill(base16, "ones64", ones64)
    fill(base16, "mask4", mask4)
    fill(base16, "xm", np.ascontiguousarray(x[m].reshape(KC, P).T))
    fill(base16, "wfb", _chunked(np.ascontiguousarray(wf_bot)))
    fill(base16, "bias3s", bias3s)
    fill(base16, "one11", np.ones((1, 1), np.float32))

    in_maps = []
    for h in range(NCORES):
        a = base16.copy()
        wsd = np.zeros((P, KC * 2), np.float32)
        for c in range(KC):
            wsd[:, c * 2 + 0] = Ws1[c * P:(c + 1) * P, h]
            wsd[:, c * 2 + 1] = Wd1[c * P:(c + 1) * P, h]
        fill(a, "wsd", wsd)
        w2f4 = np.concatenate(
            [w2fold[h * OUT:(h + 1) * OUT],
             Ws2[h * OUT:(h + 1) * OUT, None],
             Wd2[h * OUT:(h + 1) * OUT, None]], axis=1)     # [768, 4]
        fill(a, "w2f", _chunked(np.ascontiguousarray(w2f4)))
        w1h = np.ascontiguousarray(W1[:, h * OUT:(h + 1) * OUT]) * W1SCALE
        w1c = _chunked(w1h).astype(np_fp8)
        in_maps.append({
            "cst16": a.astype(np_bf16),
            "w1a": np.ascontiguousarray(w1c[:, 0:(KC - 1) * OUT]),
            "w1b": np.ascontiguousarray(w1c[:, (KC - 1) * OUT:KC * OUT]),
        })

    tail = dict(
        v1n=v1n, m=m, v1row={v: r for r, v in enumerate(v1)},
        s1_src=host["s1_src"],
        colsum2=w2fold.sum(axis=0).astype(np.float64),
        sws2=float(Ws2.sum()), swd2=float(Wd2.sum()),
    )
    return meta, in_maps, tail


def make_in_maps(**inputs):
    meta, in_maps, _ = _prepare(**inputs)
    return meta, in_maps


def _host_tail(tail, h2f, oxm):
    """Layer-2 segment softmax over mask's in-edges + classifier add."""
    v1row, m = tail["v1row"], tail["m"]
    vs = h2f[:, 0:2] - tail["colsum2"]            # helu'-1 fold
    a2s = h2f[:, 2] - tail["sws2"]
    a2d_m = h2f[v1row[m], 3] - tail["swd2"]
    lg = np.array([a2s[v1row[s]] for s in tail["s1_src"]], np.float64) + a2d_m
    lg = np.where(lg > 0, lg, 0.2 * lg)
    e = np.exp(lg - lg.max())
    alpha = e / (e.sum() + 1e-16)
    h2top = alpha @ np.stack([vs[v1row[s]] for s in tail["s1_src"]])
    return (h2top + oxm).reshape(1, 2).astype(np.float32)


def kernel(**inputs):
    meta, in_maps, tail = _prepare(**inputs)
    nc = _get_nc(meta)
    res = run_bass_kernel_spmd(nc, in_maps, core_ids=list(range(NCORES)))
    v1n = meta["v1n"]
    parts = [np.asarray(r["res"], np.float64).reshape(4 * v1n + 2)
             for r in res.results]
    h2f = sum(p[0:4 * v1n] for p in parts).reshape(v1n, 4)
    oxm = parts[0][4 * v1n:4 * v1n + 2]
    return _host_tail(tail, h2f, oxm)
